# revision 54
# baseline (speedup 1.0000x reference)
"""DivergentAttention Trainium2 kernel (8 NeuronCores, Bass/Tile), v3.

Problem: GPT-2 style causal self-attention (B=2, S=2048, D=1024, H=16,
hd=64) where heads 0/1/2 re-weight their attention toward a token region
(first/middle/last third of the sequence) with factor 1.6 and renormalize.

Identity: softmax(s)*m / sum(softmax(s)*m) == softmax(s + log m): the region
reweight folds into an additive per-(head, key) bias on the scores. Scores
are small (|s|<~5) so the max-subtraction pass is skipped.

Sharding: core c handles batch c//4 and heads [4*(c%4), 4*(c%4)+4); host
sums the 8 c_proj partials (bf16) and adds c_proj_b.

Design (~107us cost-model makespan vs 151us baseline; hw rel err 7.6e-3):
  - QKV projection in fp8e4m3 DoubleRow (0.5 cyc/col), 3-term hi/lo
    error compensation: (w_hi+w_lo)(h_hi+h_lo) minus the negligible
    lo*lo term; 12 DoubleRow matmuls contract all 1024 rows vs 8 bf16
    ones. Weights are host-prescaled by 64 (fp8e4m3 subnormal cutoff);
    q/k copies descale via DVE tensor_scalar(mult, add), and the v-path
    descale cancels in the softmax normalization by setting the
    denominator ones-column to 64.
  - Scores in fp8 DoubleRow too: q/k stripes each followed by a ZEROED
    stripe so both slot-1 operands are benign (k-side weights are zero,
    and the q-side data is never an uninitialized-SBUF NaN pattern).
  - AV is FLIPPED: out[q-tile 128, 65] = attnT_tile.T @ [v | 1]: 65
    moving cols per (q,k) tile pair halves AV cost vs the [65, q-width]
    orientation, and the denominator (col 64) lands on the same
    partitions as q, so normalization is a per-partition reciprocal +
    tensor_scalar_mul on DVE (no partition broadcast).
  - Per-(head,q-tile) accumulators are packed 7-per-PSUM-bank at 65*4B
    stride. Hardware PSUM accumulation groups do NOT interleave within
    a bank, so the bank is DVE-memset to zero and all AV matmuls use
    start=False (plain accumulate). Banks drain only once quiescent
    (reading a bank mid-accumulation corrupts concurrent writes).
  - ao[q, hd] is DMA-xbar-transposed to aoT[hd, q] per (q-tile,
    head-pair) for c_proj, which is interleaved into the phase-2 tail.
  - ACT runs (almost) only the exp stream, in [128, <=1024] pieces
    straight out of score PSUM; all other PSUM->SBUF copies are on DVE.
    GPSIMD applies the causal 0/1 mask post-exp (all-SBUF) and issues
    SWDGE DMAs. GPSIMD cannot touch PSUM (BIR verifier).
  - Emission order IS the dependency order (Tile derives deps from
    program order) and engine-queue priority: qk groups, v tiles and
    attention pieces are interleaved so the exp stream starts ~7us in
    and every v_tile(t) precedes the first tail that reads it (build
    asserts this invariant).
"""

import numpy as np

import concourse.bass as bass
import concourse.tile as tile
from concourse import mybir
from concourse import bass_utils, bass2jax

# ---------------------------------------------------------------- constants
B, S, D, H, HD = 2, 2048, 1024, 16, 64
NCORES = 8
HPC = 4              # heads per core
GROUPS = 4           # head groups
FOCUS = 1.6
HEAD_REGION = {0: 0, 1: 1, 2: 2}
BF = mybir.dt.bfloat16
F32 = mybir.dt.float32
F8 = mybir.dt.float8e4
NT = S // 128         # 16
KO = D // 128         # 8
CP = 4                # DoubleRow chunk-pairs (256 logical rows each)
WSCALE = 64.0         # c_attn_w prescale so fp8e4m3 stays out of subnormals
DEBUG_DUMPS = False   # add intermediate-tensor DRAM dumps (debugging only)
# drains are bank-granular: reading a PSUM bank while matmuls still
# accumulate into other columns of it corrupts the accumulation (hw
# read-during-accumulate hazard), so a bank drains only once quiescent.
BATCHES = ((0, 7), (7, 14), (14, 16))
DRAIN_T = {6: 0, 13: 1, 15: 2}               # tail t -> batch index
BANK0 = (0, 7, 14)
LAST_IN_BANK = (True, True, True)

# ------------------------------------------------- walrus multi-wait fixup
# This container's walrus accepts only ONE sync-wait per TPB instruction,
# but Tile attaches one wait per dependency proc. Rewrite the BIR JSON just
# before walrus: hoist all-but-one wait of a multi-wait instruction onto
# standalone same-engine NoOps inserted immediately before it (same-engine
# program order is preserved, so semantics are unchanged).
try:
    import orjson as _json
except ImportError:  # pragma: no cover
    import json as _json

_orig_compile_bir_kernel = bass_utils.compile_bir_kernel
_wfix_counter = [0]


def _fix_bir(bir_json):
    d = _json.loads(bir_json)
    changed = False
    for fn in d.get("functions", []):
        for blk in fn.get("blocks", []):
            out = []
            for inst in blk.get("instructions", []):
                si = inst.get("sync_info")
                if si:
                    waits = si.get("on_wait") or []
                    if len(waits) > 1:
                        changed = True
                        for w in waits[:-1]:
                            _wfix_counter[0] += 1
                            nop = {
                                "engine": inst["engine"],
                                "ins": [],
                                "name": f"I-wfix-{_wfix_counter[0]}",
                                "opcode": "NoOp",
                                "outs": [],
                                "sync_info": {"on_update": [], "on_wait": [w]},
                            }
                            if "debug" in inst:
                                nop["debug"] = inst["debug"]
                            out.append(nop)
                        si["on_wait"] = waits[-1:]
                out.append(inst)
            blk["instructions"] = out
    return _json.dumps(d) if changed else bir_json


def _patched_compile_bir_kernel(bir_json, tmpdir, neff_name="file.neff"):
    return _orig_compile_bir_kernel(_fix_bir(bir_json), tmpdir, neff_name=neff_name)


def _install_waitfix():
    bass_utils.compile_bir_kernel = _patched_compile_bir_kernel
    bass2jax.compile_bir_kernel = _patched_compile_bir_kernel


_install_waitfix()

# ---------------------------------------------------------------- program


def build_program():
    """One SPMD Bass program; per-core differences come in via inputs."""
    nc = bass.Bass()

    # hi/lo fp8 split of hiddenT and 64*c_attn_w, DoubleRow-packed:
    # [p, cp, slot, col] holds logical contraction row 256*cp + 128*slot + p.
    hp_hi = nc.dram_tensor("hp_hi", [128, CP, 2, S], F8, kind="ExternalInput")
    hp_lo = nc.dram_tensor("hp_lo", [128, CP, 2, S], F8, kind="ExternalInput")
    wp_hi = nc.dram_tensor("wp_hi", [128, CP, 2, 768], F8, kind="ExternalInput")
    wp_lo = nc.dram_tensor("wp_lo", [128, CP, 2, 768], F8, kind="ExternalInput")
    bqk = nc.dram_tensor("bqk", [128, 4], F32, kind="ExternalInput")
    bv_rep = nc.dram_tensor("bv_rep", [128, 256], F32, kind="ExternalInput")
    projw = nc.dram_tensor("projw", [128, 2, D], BF, kind="ExternalInput")
    diag_mask = nc.dram_tensor("diag_mask", [128, 128], BF, kind="ExternalInput")
    logmult = nc.dram_tensor("logmult", [128, HPC, NT], F32, kind="ExternalInput")
    out = nc.dram_tensor("out", [S, D], BF, kind="ExternalOutput")
    if DEBUG_DUMPS:
        dbg_qk = nc.dram_tensor("dbg_qk", [128, 8, S], F8, kind="ExternalOutput")
        dbg_v = nc.dram_tensor("dbg_v", [128, NT, HPC, 65], BF,
                               kind="ExternalOutput")
        dbg_ao = nc.dram_tensor("dbg_ao", [128, NT, 2, 128], BF,
                                kind="ExternalOutput")
        dbg_aoT = nc.dram_tensor("dbg_aoT", [128, NT, 2, 128], BF,
                                 kind="ExternalOutput")
        dbg_at = nc.dram_tensor("dbg_at", [128, 4, 512], BF,
                                kind="ExternalOutput")
        dbg_av = nc.dram_tensor("dbg_av", [128, 455], F32,
                                kind="ExternalOutput")

    with tile.TileContext(nc) as tc:
        with tc.tile_pool(name="persist", bufs=1) as persist, \
             tc.tile_pool(name="p2at", bufs=20) as p2at, \
             tc.tile_pool(name="p2rec", bufs=4) as p2rec, \
             tc.tile_pool(name="p2o", bufs=6) as p2o, \
             tc.tile_pool(name="p2sc", bufs=2, space="PSUM") as p2sc, \
             tc.tile_pool(name="p2av", bufs=2, space="PSUM") as p2av, \
             tc.tile_pool(name="mix", bufs=2, space="PSUM") as mix:

            # fp8 q/k for DoubleRow scores: each stripe is followed by a
            # ZEROED stripe so both DoubleRow slot-1 operands are benign:
            # the k-side slot-1 weights are 0.0 and the q-side slot-1 data
            # is 0.0 (never a NaN bit pattern from uninitialized SBUF).
            # snt: q01=0, q23=2, k01=4, k23=6; zeros at 1, 3, 5, 7.
            qk_sb = persist.tile([128, 8, S], F8)            # 2 MB
            v_sb = persist.tile([128, NT, HPC, 65], BF)      # ~1.06 MB
            ao_sb = persist.tile([128, NT, 2, 128], BF)      # 1 MB
            aoT_sb = persist.tile([128, NT, 2, 128], BF)     # 1 MB
            bqk_sb = persist.tile([128, 4], F32)
            bv_sb = persist.tile([128, 256], F32)
            pw_sb = persist.tile([128, 2, D], BF)
            dm_sb = persist.tile([128, 128], BF)
            lm_sb = persist.tile([128, HPC, NT], F32)
            hhi = persist.tile([128, CP, 2, S], F8)          # 2 MB
            hlo = persist.tile([128, CP, 2, S], F8)          # 2 MB
            whi = persist.tile([128, CP, 2, 768], F8)
            wlo = persist.tile([128, CP, 2, 768], F8)

            # v_aug ones column = WSCALE so the w-prescale cancels in the
            # softmax normalization (num and den both carry WSCALE).
            nc.vector.memset(v_sb[:, :, :, 64:65], WSCALE)
            for z in (1, 3, 5, 7):
                nc.vector.memset(qk_sb[:, z, :].bitcast(F32), 0.0)

            # ---- loads ----
            # hidden hi parts in 1024-col chunks so the first qk groups
            # start ~3us in; col-block 1 only feeds the sc=2,3 groups.
            qs3 = (nc.sync, nc.scalar)
            for cb in range(2):
                for i, (src_t, dst_t) in enumerate(((hp_hi, hhi), (hp_lo, hlo))):
                    for cp in range(CP):
                        q = qs3[(i + cp) % 2]
                        q.dma_start(
                            dst_t[:, cp, :, 1024 * cb:1024 * cb + 1024],
                            src_t[:, cp, :, 1024 * cb:1024 * cb + 1024])
                        if cb == 0 and i == 0:
                            nc.gpsimd.dma_start(whi[:, cp, :, :],
                                                wp_hi[:, cp, :, :])
                        elif cb == 0:
                            nc.gpsimd.dma_start(wlo[:, cp, :, :],
                                                wp_lo[:, cp, :, :])
                if cb == 0:
                    nc.sync.dma_start(bqk_sb, bqk[:, :])
                    nc.scalar.dma_start(bv_sb, bv_rep[:, :])
                    nc.gpsimd.dma_start(dm_sb, diag_mask[:, :])
                    nc.gpsimd.dma_start(lm_sb, logmult[:, :, :])
                    nc.gpsimd.dma_start(pw_sb, projw[:, :, :])

            # ---- phase-1 building blocks ----
            _gq = [0]

            # 3-term hi/lo fp8 product: (whi+wlo)(hhi+hlo) minus the
            # negligible wlo*hlo term; 12 DoubleRow matmuls at 0.5 cyc/col
            # contract all 1024 rows (vs 8 matmuls at 1.0 for bf16).
            QKV_TERMS = ((whi, hhi), (wlo, hhi), (whi, hlo))

            def qk_group(nt, sc, eng):
                # one [128, 512] output group; the PSUM->SBUF copy descales
                # by 1/WSCALE and adds the per-partition bias.
                _gq[0] += 1
                ps = mix.tile([128, 512], F32, tag="m", name=f"qk{_gq[0]}")
                n = len(QKV_TERMS) * CP
                i = 0
                for wt, ht in QKV_TERMS:
                    for cp in range(CP):
                        nc.tensor.matmul(
                            ps,
                            wt[:, cp, :, 128 * nt:128 * nt + 128],
                            ht[:, cp, :, 512 * sc:512 * sc + 512],
                            start=(i == 0), stop=(i == n - 1),
                            perf_mode=mybir.MatmulPerfMode.DoubleRow,
                        )
                        i += 1
                snt = (0, 2, 4, 6)[nt]   # storage stripe (zeros odd)
                eng.tensor_scalar(
                    qk_sb[:, snt, 512 * sc:512 * sc + 512],
                    ps,
                    1.0 / WSCALE,
                    bqk_sb[:, nt:nt + 1],
                    mybir.AluOpType.mult,
                    mybir.AluOpType.add,
                )

            v_emitted = set()

            def v_tile(st):
                v_emitted.add(st)
                ps = mix.tile([128, 512], F32, tag="m", name=f"v{st}")
                n = len(QKV_TERMS) * CP
                i = 0
                for wt, ht in QKV_TERMS:
                    for cp in range(CP):
                        nc.tensor.matmul(
                            ps[:, 0:256],
                            ht[:, cp, :, 128 * st:128 * st + 128],
                            wt[:, cp, :, 512:768],
                            start=(i == 0), stop=(i == n - 1),
                            perf_mode=mybir.MatmulPerfMode.DoubleRow,
                        )
                        i += 1
                # GPSIMD cannot read PSUM (BIR verifier) -> DVE
                nc.vector.tensor_add(
                    out=v_sb[:, st, :, 0:64],
                    in0=ps[:, 0:256].rearrange("p (h d) -> p h d", d=64),
                    in1=bv_sb.rearrange("p (h d) -> p h d", d=64),
                )

            # ---- phase-2 building blocks ----
            av_tiles = {}

            def get_av(lh, tau):
                # hw PSUM accumulation groups don't interleave within a
                # bank: pre-zero the bank and accumulate with start=False.
                bank = 0 if tau < 7 else (1 if tau < 14 else 2)
                if (lh, bank) not in av_tiles:
                    pool = p2av if bank < 2 else mix
                    tag = "av" if bank < 2 else "m"
                    tile_ = pool.tile(
                        [128, 512], F32, tag=tag, name=f"av{lh}{bank}")
                    nc.vector.memset(tile_[:, :], 0.0)
                    av_tiles[(lh, bank)] = tile_
                return av_tiles[(lh, bank)], 7 * (bank > 0) + 7 * (bank > 1)

            def cproj(tau):
                for ec in range(2):
                    # tau>=7 c_proj runs after h3's av batches 0/1 freed
                    # their p2av slots, so it can use them for parallelism.
                    if tau < 7:
                        pool, tg = mix, "m"
                    elif (2 * tau + ec) % 3 == 0:
                        pool, tg = mix, "m"
                    else:
                        pool, tg = p2av, "av"
                    ps = pool.tile([128, 512], F32, tag=tg, name=f"pr{tau}{ec}")
                    for j in range(2):
                        nc.tensor.matmul(
                            ps,
                            aoT_sb[:, tau, j, :],
                            pw_sb[:, j, 512 * ec:512 * ec + 512],
                            start=(j == 0), stop=(j == 1),
                        )
                    o_sb = p2o.tile([128, 512], BF, tag="o")
                    # GPSIMD cannot read PSUM; keep ACT clear for the exp
                    # stream -> all c_proj copies on DVE.
                    if ec == 1 and tau >= 10:
                        nc.scalar.copy(o_sb, ps)
                    else:
                        nc.vector.tensor_copy(o_sb, ps)
                    if tau >= 13:
                        oq = (nc.sync, nc.gpsimd, nc.scalar)[(2 * tau + ec) % 3]
                    else:
                        oq = nc.sync if ec == 0 else nc.gpsimd
                    oq.dma_start(
                        out[128 * tau:128 * tau + 128,
                            512 * ec:512 * ec + 512],
                        o_sb,
                    )

            def drain(lh, b):
                # all q-tiles of this batch fully accumulated: reciprocal
                # of the denominator column, per-partition scale into ao_sb.
                t0, t1 = BATCHES[b]
                nb = t1 - t0
                bank = 0 if t0 < 7 else (1 if t0 < 14 else 2)
                if LAST_IN_BANK[b]:
                    av = av_tiles.pop((lh, bank))
                else:
                    av = av_tiles[(lh, bank)]
                av = av[:, 65 * (t0 - BANK0[b]):]
                j, hp = lh // 2, lh % 2
                if DEBUG_DUMPS and lh == 0 and b == 0:
                    avcp = persist.tile([128, 455], F32)
                    nc.vector.tensor_copy(avcp, av[:, 0:455])
                    nc.sync.dma_start(dbg_av[:, :], avcp[:, :])
                rec = p2rec.tile([128, 8], F32, tag="rec")
                den = av[:, 0:65 * nb].rearrange(
                    "p (n c) -> p n c", c=65)[:, :, 64:65]
                nc.vector.reciprocal(rec[:, 0:nb], den)
                for k in range(nb):
                    tau = t0 + k
                    nc.vector.tensor_scalar_mul(
                        ao_sb[:, tau, j, 64 * hp:64 * hp + 64],
                        av[:, 65 * k:65 * k + 64],
                        rec[:, k:k + 1],
                    )
                if hp == 1:
                    # both heads of pair j drained: transpose ao[q, hd] ->
                    # aoT[hd, q] on the DMA xbar; after the last pair,
                    # this q-tile's c_proj is fully unblocked.
                    for k in range(nb):
                        tau = t0 + k
                        nc.sync.dma_start_transpose(
                            aoT_sb[:, tau, j, :], ao_sb[:, tau, j, :])
                    if lh == 3:
                        for k in range(nb):
                            cproj(t0 + k)

            def tail(lh, t, q0, width, at_sb):
                # exp consumers: causal 0/1 mask on the diagonal block
                # (GPSIMD, all-SBUF) + flipped AV accumulation.
                if q0 == 128 * t:
                    nc.gpsimd.tensor_mul(
                        out=at_sb[:, 0:128], in0=at_sb[:, 0:128],
                        in1=dm_sb,
                    )
                if DEBUG_DUMPS and lh == 0 and t < 4 and q0 < 512:
                    nc.sync.dma_start(dbg_at[:, t, :], at_sb[:, 0:512])
                assert t in v_emitted, (
                    f"tail({lh},{t}) before v_tile({t}): program-order "
                    "dependency violation (reads uninitialized v_sb)")
                v_aug = v_sb[:, t, lh, :]
                for tau in range(q0 // 128, (q0 + width) // 128):
                    av, base = get_av(lh, tau)
                    col = 65 * (tau - base)
                    off = 128 * tau - q0
                    nc.tensor.matmul(
                        av[:, col:col + 65],
                        at_sb[:, off:off + 128],
                        v_aug,
                        start=False, stop=(t == tau),
                        skip_group_check=True,
                    )
                full = q0 + width == (1024 if t < 8 else 2048)
                if full and t in DRAIN_T and (t > 6 or q0 < 1024):
                    drain(lh, DRAIN_T[t])

            pending = []

            def piece(lh, t, hf, q0=None, q1=None):
                if q0 is None:
                    q0 = max(128 * t, 1024 * hf)
                if q1 is None:
                    q1 = 1024 * (hf + 1)
                if q0 >= q1:
                    return
                width = q1 - q0
                bp = 64 * (lh % 2)
                q_nt = 2 * (lh // 2)          # slots (q stripe, zeros)
                k_nt = 4 + 2 * (lh // 2)      # slots (k stripe, zeros)
                lhsT_k = qk_sb[bp:bp + 64, k_nt:k_nt + 2,
                               128 * t:128 * t + 128]
                sc_ps = p2sc.tile([128, 1024], F32, tag="sc")
                off = 0
                while off < width:
                    w512 = min(512, width - off)
                    nc.tensor.matmul(
                        sc_ps[:, off:off + w512],
                        lhsT_k,
                        qk_sb[bp:bp + 64, q_nt:q_nt + 2,
                              q0 + off:q0 + off + w512],
                        start=True, stop=True,
                        perf_mode=mybir.MatmulPerfMode.DoubleRow,
                    )
                    off += w512
                at_sb = p2at.tile([128, 1024], BF, tag="attnT")
                nc.scalar.activation(
                    at_sb[:, :width], sc_ps[:, :width],
                    mybir.ActivationFunctionType.Exp,
                    bias=lm_sb[:, lh, t:t + 1], scale=0.125,
                )
                pending.append((lh, t, q0, width, at_sb))
                if len(pending) > 6:
                    tail(*pending.pop(0))

            # ---- interleaved emission: program order is engine priority ----
            V = nc.vector
            P = nc.gpsimd
            # NOTE: tails (av matmuls) consume v_sb, and Tile derives
            # dependencies from program order -- every v_tile(st) must be
            # emitted BEFORE the first tail that reads v_sb[:, st].
            # With pending depth 6, tail of piece i pops at piece i+7.
            qk_group(2, 0, V)                 # k01 cols 0:512
            qk_group(0, 0, V)                 # q01 cols 0:512
            for t in range(4):
                piece(0, t, 0, q1=512)        # needs only the two groups above
            v_tile(0)
            v_tile(1)
            qk_group(0, 1, V)                 # q01 cols 512:1024
            for t in range(4):
                piece(0, t, 0, q0=512)
            v_tile(2)
            v_tile(3)
            qk_group(2, 1, V)
            for t in range(4, 8):
                piece(0, t, 0)
            qk_group(0, 2, V)
            qk_group(0, 3, V)
            for st in range(4, 8):
                v_tile(st)
            for t in range(6):
                piece(0, t, 1)
            qk_group(2, 2, V)
            qk_group(2, 3, V)
            for t in range(6, 10):
                piece(0, t, 1)
            for st in range(8, 11):
                v_tile(st)
            for t in range(10, 16):
                piece(0, t, 1)
            for st in range(11, 16):
                v_tile(st)
            for t in range(8):
                piece(1, t, 0)
            for t in range(16):
                piece(1, t, 1)
            qk_group(1, 0, V)                 # q23 cols 0:512
            qk_group(3, 0, V)                 # k23 cols 0:512
            qk_group(1, 1, V)
            qk_group(3, 1, V)
            for t in range(8):
                piece(2, t, 0)
            qk_group(1, 2, V)
            qk_group(3, 2, V)
            qk_group(1, 3, V)
            qk_group(3, 3, V)
            for t in range(16):
                piece(2, t, 1)
            for t in range(8):
                piece(3, t, 0)
            for t in range(16):
                piece(3, t, 1)
            for pc in pending:
                tail(*pc)
            pending.clear()
            if DEBUG_DUMPS:
                nc.sync.dma_start(dbg_qk[:, :, :], qk_sb[:, :, :])
                nc.sync.dma_start(dbg_v[:, :, :, :], v_sb[:, :, :, :])
                nc.sync.dma_start(dbg_ao[:, :, :, :], ao_sb[:, :, :, :])
                nc.sync.dma_start(dbg_aoT[:, :, :, :], aoT_sb[:, :, :, :])
    return nc


_NC = None


def _get_nc():
    global _NC
    if _NC is None:
        _NC = build_program()
    return _NC


# ---------------------------------------------------------------- host prep

def make_in_maps(hidden_states, c_attn_w, c_attn_b, c_proj_w):
    import ml_dtypes
    bf16 = ml_dtypes.bfloat16
    f8 = mybir.dt.np(F8)

    def pack_hilo(arr):
        # [1024, N] f32 -> hi/lo fp8 DoubleRow packs [128, CP, 2, N]
        hi = arr.astype(f8)
        lo = (arr - hi.astype(np.float32)).astype(f8)
        out = []
        for part in (hi, lo):
            p = part.reshape(CP, 2, 128, -1).transpose(2, 0, 1, 3)
            out.append(np.ascontiguousarray(p))
        return out

    first_end = S // 3
    second_end = 2 * S // 3
    pos = np.arange(S)
    regions = [pos < first_end,
               (pos >= first_end) & (pos < second_end),
               pos >= second_end]
    mult = np.ones((H, S), dtype=np.float64)
    for h, r in HEAD_REGION.items():
        mult[h] = 1.0 + (FOCUS - 1.0) * regions[r].astype(np.float64)
    logm = np.log(mult).astype(np.float32)  # [H, S]

    p = np.arange(128)[:, None]
    j = np.arange(128)[None, :]
    diag = (j >= p).astype(np.float32)  # 0/1 keep-mask, applied post-exp

    in_maps = []
    for c in range(NCORES):
        b, g = divmod(c, GROUPS)
        h0 = HPC * g
        cs = slice(256 * g, 256 * g + 256)
        w_qkv = np.concatenate(
            [c_attn_w[:, cs], c_attn_w[:, 1024:2048][:, cs],
             c_attn_w[:, 2048:3072][:, cs]], axis=1,
        )
        bqk = np.concatenate(
            [c_attn_b[cs], c_attn_b[1024:2048][cs]]
        ).reshape(4, 128).T.copy().astype(np.float32)
        bv = WSCALE * np.broadcast_to(
            c_attn_b[2048:3072][cs], (128, 256)
        ).astype(np.float32)
        # pw2[p, j, e]: head pair j=(2j, 2j+1); p<64 -> head 2j row p,
        # p>=64 -> head 2j+1 row p-64  (matches aoT partition layout)
        pw = c_proj_w[64 * h0:64 * h0 + 256, :].reshape(2, 128, D)
        pw = np.ascontiguousarray(pw.transpose(1, 0, 2))
        lm = logm[h0:h0 + HPC].reshape(HPC, S // 128, 128)
        lm = np.ascontiguousarray(lm.transpose(2, 0, 1)).astype(np.float32)
        h_hi, h_lo = pack_hilo(np.ascontiguousarray(hidden_states[b].T))
        w_hi, w_lo = pack_hilo(WSCALE * w_qkv)
        in_maps.append({
            "hp_hi": h_hi,
            "hp_lo": h_lo,
            "wp_hi": w_hi,
            "wp_lo": w_lo,
            "bqk": bqk,
            "bv_rep": bv,
            "projw": pw.astype(bf16),
            "diag_mask": diag.astype(bf16),
            "logmult": lm,
        })
    return in_maps


def run_cores(in_maps, trace=False, **kw):
    from concourse.bass_utils import run_bass_kernel_spmd
    nc = _get_nc()
    return run_bass_kernel_spmd(nc, in_maps, core_ids=list(range(NCORES)),
                                trace=trace, **kw)


def kernel(hidden_states, c_attn_w, c_attn_b, c_proj_w, c_proj_b):
    hidden_states = np.asarray(hidden_states, dtype=np.float32)
    c_attn_w = np.asarray(c_attn_w, dtype=np.float32)
    c_attn_b = np.asarray(c_attn_b, dtype=np.float32)
    c_proj_w = np.asarray(c_proj_w, dtype=np.float32)
    c_proj_b = np.asarray(c_proj_b, dtype=np.float32)

    in_maps = make_in_maps(hidden_states, c_attn_w, c_attn_b, c_proj_w)
    res = run_cores(in_maps)
    out = np.zeros((B, S, D), dtype=np.float32)
    for c in range(NCORES):
        out[c // GROUPS] += np.asarray(res.results[c]["out"], dtype=np.float32)
    out += c_proj_b[None, None, :]
    return out
